# revision 17
# baseline (speedup 1.0000x reference)
"""Slot-attention module Bass/Tile kernel (nn_AttentionModule_39084202394083).

Contract: kernel(**inputs) takes FULL unsharded inputs (B=64, N=4096, D=256,
S=8 slots, 3 iterations) and returns the FULL output [S=8, B=64, D=256] f32.

Sharding: data-parallel over batch B across 8 NeuronCores (8 batch elements
per core, all params replicated). Each core runs an identical Bass program on
its batch slice; no collectives.

Per-core program design:
  - LN(inputs) affine params (g_in/b_in) are folded into the k/v projection
    weights on device; LN on device is pure (x-mean)*rstd (stats via
    bn_stats, normalize+bf16-cast via tensor_scalar on GPSIMD).
  - x^T (needed because matmul contracts over the partition dim) is produced
    with SBUF->SBUF DMA xbar transposes of bf16 tiles.
  - k is kept transposed [D, N] (weight-stationary matmuls); v is kept
    natural [N, D] with an appended ones-column so the per-slot attention
    normalizer falls out of the updates matmul for free.
  - dots are computed directly transposed [N, 8] (k^T-chunk-stationary
    matmuls) so softmax over slots is a free-dim reduce; exp folds the
    1/sqrt(D) scale; the per-position softmax denominator is applied to E^T
    via a free-dim-broadcast tensor_tensor multiply.
  - Slot-side GEMMs (q/GRU/MLP) are batched over 4 batch elements (32 rows)
    per round with shared weights; biases enter as K=1 ones-row matmuls or
    per-partition activation biases. GRU gates/LN run in fp32.
  - Batches are processed in 2 rounds of 4 so bf16 k/v for a round fits SBUF.
"""

import os
import numpy as np
from contextlib import ExitStack

import concourse.bass as bass
import concourse.tile as tile
from concourse import bacc
from concourse import mybir
from concourse.bass import ts
from concourse.bass_utils import run_bass_kernel_spmd
from concourse.masks import make_identity

F32 = mybir.dt.float32
BF16 = mybir.dt.bfloat16
AF = mybir.ActivationFunctionType
ALU = mybir.AluOpType

B_LOC = 8          # batch elements per core
N = 4096           # positions
D = 256            # model dim
S = 8              # slots
H = 1024           # mlp hidden
NT = N // 128      # 32 position tiles
KD = 2             # 128-chunks of D
KH = H // 128      # 8 128-chunks of H
ITERS = 3
ROUND = 4          # batches per round
NROUND = B_LOC // ROUND
SUP = 4            # position tiles per load super-tile
LN_EPS = 1e-5
SCALE = float(D) ** -0.5

W_NAMES = [
    ("wq", [D, D]), ("bq", [D]), ("wk", [D, D]), ("bk", [D]),
    ("wv", [D, D]), ("bv", [D]),
    ("w_ih", [D, 3 * D]), ("b_ih", [3 * D]),
    ("w_hh", [D, 3 * D]), ("b_hh", [3 * D]),
    ("mlp_w1", [D, H]), ("mlp_b1", [H]),
    ("mlp_w2", [H, D]), ("mlp_b2", [D]),
    ("g_in", [D]), ("b_in", [D]), ("g_sl", [D]), ("b_sl", [D]),
    ("g_ff", [D]), ("b_ff", [D]),
]


def _build():
    nc = bacc.Bacc("TRN2", debug=False, enable_asserts=False)
    inp = nc.dram_tensor("inputs", [B_LOC, N, D], F32, kind="ExternalInput").ap()
    slots_in = nc.dram_tensor("slots", [S, B_LOC, D], F32, kind="ExternalInput").ap()
    W = {}
    for name, shape in W_NAMES:
        W[name] = nc.dram_tensor(name, shape, F32, kind="ExternalInput").ap()
    out_dram = nc.dram_tensor("out", [S, B_LOC, D], F32, kind="ExternalOutput").ap()

    with tile.TileContext(nc) as tc:
        with ExitStack() as ctx:
            _body(ctx, tc, inp, slots_in, W, out_dram)
    nc.compile()
    return nc


def _body(ctx, tc, inp, slots_in, W, out_dram):
    nc = tc.nc

    wts = ctx.enter_context(tc.tile_pool(name="wts", bufs=1))
    ps = ctx.enter_context(tc.tile_pool(name="ps", bufs=2, space="PSUM"))

    # ---------------- persistent constants / folded weights ----------------
    id_f = wts.tile([128, 128], F32, tag="idf")
    make_identity(nc, id_f)
    id_b = wts.tile([128, 128], BF16, tag="idb")
    make_identity(nc, id_b)
    ones_row = wts.tile([1, 64], BF16, tag="ones_row")
    nc.vector.memset(ones_row, 1.0)
    eps_col = wts.tile([128, 1], F32, tag="eps_col")
    nc.vector.memset(eps_col, LN_EPS)

    wkpb = wts.tile([128, KD, D], BF16, tag="wkpb")
    wvpb = wts.tile([128, KD, D], BF16, tag="wvpb")
    wqpb = wts.tile([128, KD, D], BF16, tag="wqpb")
    wihb = wts.tile([128, KD, 3 * D], BF16, tag="wihb")
    whhb = wts.tile([128, KD, 3 * D], BF16, tag="whhb")
    w1pb = wts.tile([128, KD, H], BF16, tag="w1pb")
    w2b = wts.tile([128, KH, D], BF16, tag="w2b")
    bkp_col = wts.tile([128, KD, 1], F32, tag="bkp")   # b_in@wk + bk, as column
    bqp_col = wts.tile([128, KD, 1], F32, tag="bqp")   # b_sl@wq + bq, as column
    gxbias_row = wts.tile([1, 3 * D], BF16, tag="gxbias")  # (bv')@w_ih + b_ih
    ghbias_row = wts.tile([1, 3 * D], BF16, tag="ghbias")  # b_hh
    b1p_row = wts.tile([1, H], BF16, tag="b1p")        # b_ff@mlp_w1 + mlp_b1
    b2_row = wts.tile([1, D], BF16, tag="b2")          # mlp_b2

    with tc.tile_pool(name="prep", bufs=2) as prep:
        def load_col(name, n128):
            t = prep.tile([128, n128, 1], F32, tag="col_" + name)
            for j in range(n128):
                nc.sync.dma_start(out=t[:, j, :],
                                  in_=W[name][ts(j, 128)].rearrange("(a one) -> a one", one=1))
            return t

        def load_row(name, n):
            t = prep.tile([1, n], F32, tag="row_" + name)
            nc.sync.dma_start(out=t, in_=W[name].rearrange("(one a) -> one a", one=1))
            return t

        def load_mat(name, rows, cols):
            t = prep.tile([128, rows // 128, cols], F32, tag="mat")
            for j in range(rows // 128):
                nc.sync.dma_start(out=t[:, j, :], in_=W[name][ts(j, 128), :])
            return t

        gin = load_col("g_in", KD)
        bin_ = load_col("b_in", KD)
        gsl = load_col("g_sl", KD)
        bsl = load_col("b_sl", KD)
        gff = load_col("g_ff", KD)
        bff = load_col("b_ff", KD)
        bv_col = load_col("bv", KD)

        def fold_proj(wname, bname, ln_g, ln_b, wout, bout):
            # wout = diag(ln_g) @ w (bf16);  bout = ln_b @ w + b  (column layout)
            wf = load_mat(wname, D, D)
            bcol = load_col(bname, KD)
            for dt in range(KD):
                pcol = ps.tile([128, 1], F32, tag="small")
                for kd in range(KD):
                    nc.tensor.matmul(pcol, lhsT=wf[:, kd, ts(dt, 128)],
                                     rhs=ln_b[:, kd, :],
                                     start=(kd == 0), stop=(kd == KD - 1))
                nc.vector.tensor_tensor(out=bout[:, dt, :], in0=pcol,
                                        in1=bcol[:, dt, :], op=ALU.add)
            for kd in range(KD):
                nc.vector.tensor_scalar(out=wout[:, kd, :], in0=wf[:, kd, :],
                                        scalar1=ln_g[:, kd, :], scalar2=None,
                                        op0=ALU.mult)

        fold_proj("wk", "bk", gin, bin_, wkpb, bkp_col)
        fold_proj("wq", "bq", gsl, bsl, wqpb, bqp_col)
        # wv: fold g_in; bias column bv' = b_in@wv + bv is folded into the GRU
        # gx bias row instead of into stored v.
        bvp_col = prep.tile([128, KD, 1], F32, tag="bvp")
        wvf = load_mat("wv", D, D)
        for dt in range(KD):
            pcol = ps.tile([128, 1], F32, tag="small")
            for kd in range(KD):
                nc.tensor.matmul(pcol, lhsT=wvf[:, kd, ts(dt, 128)],
                                 rhs=bin_[:, kd, :],
                                 start=(kd == 0), stop=(kd == KD - 1))
            nc.vector.tensor_tensor(out=bvp_col[:, dt, :], in0=pcol,
                                    in1=bv_col[:, dt, :], op=ALU.add)
        for kd in range(KD):
            nc.vector.tensor_scalar(out=wvpb[:, kd, :], in0=wvf[:, kd, :],
                                    scalar1=gin[:, kd, :], scalar2=None,
                                    op0=ALU.mult)

        # GRU weights (no LN folding) + bias rows.
        wihf = load_mat("w_ih", D, 3 * D)
        for kd in range(KD):
            nc.vector.tensor_copy(out=wihb[:, kd, :], in_=wihf[:, kd, :])
        bih_row = load_row("b_ih", 3 * D)
        for blk, w in ((0, 512), (512, 256)):
            prow = ps.tile([1, 512], F32, tag="small")
            for kd in range(KD):
                nc.tensor.matmul(prow[:, 0:w], lhsT=bvp_col[:, kd, :],
                                 rhs=wihf[:, kd, blk:blk + w],
                                 start=(kd == 0), stop=(kd == KD - 1))
            nc.vector.tensor_tensor(out=gxbias_row[:, blk:blk + w],
                                    in0=prow[:, 0:w],
                                    in1=bih_row[:, blk:blk + w], op=ALU.add)
        whhf = load_mat("w_hh", D, 3 * D)
        for kd in range(KD):
            nc.vector.tensor_copy(out=whhb[:, kd, :], in_=whhf[:, kd, :])
        bhh_row = load_row("b_hh", 3 * D)
        nc.vector.tensor_copy(out=ghbias_row, in_=bhh_row)

        # MLP: fold g_ff into w1; b1' = b_ff@w1 + b1.
        w1f = load_mat("mlp_w1", D, H)
        b1_row = load_row("mlp_b1", H)
        for blk in (0, 512):
            prow = ps.tile([1, 512], F32, tag="small")
            for kd in range(KD):
                nc.tensor.matmul(prow, lhsT=bff[:, kd, :],
                                 rhs=w1f[:, kd, blk:blk + 512],
                                 start=(kd == 0), stop=(kd == KD - 1))
            nc.vector.tensor_tensor(out=b1p_row[:, blk:blk + 512], in0=prow,
                                    in1=b1_row[:, blk:blk + 512], op=ALU.add)
        for kd in range(KD):
            nc.vector.tensor_scalar(out=w1pb[:, kd, :], in0=w1f[:, kd, :],
                                    scalar1=gff[:, kd, :], scalar2=None,
                                    op0=ALU.mult)
        w2f = load_mat("mlp_w2", H, D)
        nc.vector.tensor_copy(out=w2b, in_=w2f)
        b2f = load_row("mlp_b2", D)
        nc.vector.tensor_copy(out=b2_row, in_=b2f)

    # ---------------- slots: one base-0 [32, D] tile per round ----------------
    slot_pool = ctx.enter_context(tc.tile_pool(name="slot", bufs=NROUND))

    # ---------------- pools for the main pipeline ----------------
    kv = ctx.enter_context(tc.tile_pool(name="kv", bufs=ROUND))
    xtp = ctx.enter_context(tc.tile_pool(name="xtp", bufs=1))
    xload = ctx.enter_context(tc.tile_pool(name="xload", bufs=2))
    stat = ctx.enter_context(tc.tile_pool(name="stat", bufs=2))
    itp = ctx.enter_context(tc.tile_pool(name="itp", bufs=2))
    sp = ctx.enter_context(tc.tile_pool(name="sp", bufs=1))

    def phase_a(b):
        """LN + transpose + k/v projections for batch b. Returns (kT, v)."""
        kT = kv.tile([128, KD, N], BF16, tag="kT")
        v = kv.tile([128, NT, D + 1], BF16, tag="v")
        nc.vector.memset(v[:, :, D:D + 1], 1.0)
        xT = xtp.tile([128, KD, N], BF16, tag="xT")
        inp_b = inp[b].rearrange("(c p) d -> p c d", p=128)
        mv_all = stat.tile([128, NT, 2], F32, tag="mv")
        r_all = stat.tile([128, NT, 1], F32, tag="r")

        for g in range(NT // SUP):
            xs = xload.tile([128, SUP, D], F32, tag="xs")
            nc.sync.dma_start(out=xs, in_=inp_b[:, ts(g, SUP), :])
            st6 = stat.tile([128, SUP, 6], F32, tag="st6")
            for j in range(SUP):
                nc.vector.bn_stats(out=st6[:, j, :], in_=xs[:, j, :])
            for j in range(SUP):
                nc.vector.bn_aggr(out=mv_all[:, g * SUP + j, :], in_=st6[:, j, :])
            nc.scalar.activation(out=r_all[:, ts(g, SUP), :],
                                 in_=mv_all[:, ts(g, SUP), 1:2],
                                 func=AF.Sqrt, bias=eps_col, scale=1.0)
            nc.vector.reciprocal(out=r_all[:, ts(g, SUP), :],
                                 in_=r_all[:, ts(g, SUP), :])
            xb = xload.tile([128, SUP, D], BF16, tag="xb")
            for j in range(SUP):
                t = g * SUP + j
                nc.vector.tensor_scalar(out=xb[:, j, :], in0=xs[:, j, :],
                                        scalar1=mv_all[:, t, 0:1],
                                        scalar2=r_all[:, t, 0:1],
                                        op0=ALU.subtract, op1=ALU.mult)
                for kd in range(KD):
                    nc.sync.dma_start_transpose(out=xT[:, kd, ts(t, 128)],
                                                in_=xb[:, j, ts(kd, 128)])
            # k^T projection for this 512-column chunk (weight stationary).
            for dt in range(KD):
                psk = ps.tile([128, 512], F32, tag="big")
                for kd in range(KD):
                    nc.tensor.matmul(psk, lhsT=wkpb[:, kd, ts(dt, 128)],
                                     rhs=xT[:, kd, ts(g, 512)],
                                     start=(kd == 0), stop=(kd == KD - 1))
                nc.scalar.activation(out=kT[:, dt, ts(g, 512)], in_=psk,
                                     func=AF.Identity, bias=bkp_col[:, dt, :],
                                     scale=1.0)
            # v natural projection (x^T-chunk stationary), 4 position chunks.
            for j in range(SUP):
                t = g * SUP + j
                psv = ps.tile([128, D], F32, tag="small")
                for kd in range(KD):
                    nc.tensor.matmul(psv, lhsT=xT[:, kd, ts(t, 128)],
                                     rhs=wvpb[:, kd, :],
                                     start=(kd == 0), stop=(kd == KD - 1))
                if t % 2 == 0:
                    nc.vector.tensor_copy(out=v[:, t, 0:D], in_=psv)
                else:
                    nc.scalar.copy(out=v[:, t, 0:D], in_=psv)
        return kT, v

    def pe_t(dst, src, n_par, n_free, identity, col0=0):
        """dst[128, n_free//128, col0:col0+n_par] (bf16) = src[n_par, n_free].T"""
        dt_ = src.dtype
        bp = src.base_partition()
        for j in range(n_free // 128):
            pst = ps.tile([128, 64], dt_, tag="small")
            nc.tensor.transpose(out=pst[:, 0:n_par], in_=src[:, ts(j, 128)],
                                identity=identity[bp:bp + n_par, bp:bp + n_par])
            nc.vector.tensor_copy(out=dst[:, j, col0:col0 + n_par],
                                  in_=pst[:, 0:n_par])

    def slot_ln_cast(rows, out_bf, sl):
        """pure-LN of sl[0:rows] -> out_bf (bf16)."""
        st6 = sp.tile([32, 6], F32, tag="sst6")
        mv = sp.tile([32, 2], F32, tag="smv")
        rr = sp.tile([32, 1], F32, tag="srr")
        nc.vector.bn_stats(out=st6, in_=sl)
        nc.vector.bn_aggr(out=mv, in_=st6)
        nc.scalar.activation(out=rr, in_=mv[:, 1:2], func=AF.Sqrt,
                             bias=eps_col[0:rows, :], scale=1.0)
        nc.vector.reciprocal(out=rr, in_=rr)
        nc.gpsimd.tensor_scalar(out=out_bf, in0=sl, scalar1=mv[:, 0:1],
                                scalar2=rr, op0=ALU.subtract, op1=ALU.mult)

    for rnd in range(NROUND):
        slots_r = slot_pool.tile([ROUND * S, D], F32, tag="slots")
        nc.sync.dma_start(
            out=slots_r,
            in_=slots_in[:, rnd * ROUND:(rnd + 1) * ROUND, :].rearrange(
                "s b d -> b s d"))
        kTs, vs = [], []
        for bl in range(ROUND):
            kT, v = phase_a(rnd * ROUND + bl)
            kTs.append(kT)
            vs.append(v)

        for it in range(ITERS):
            # ---- batched slot-side: sn = LN(slots), qT = wq'^T @ sn^T ----
            sn = sp.tile([32, D], BF16, tag="sn")
            slot_ln_cast(32, sn, slots_r)
            snT = sp.tile([128, KD, 32], BF16, tag="snT")
            pe_t(snT, sn, 32, D, id_b)
            qT = sp.tile([128, KD, 32], BF16, tag="qT")
            psq = ps.tile([128, 64], F32, tag="small")
            for dt in range(KD):
                for kd in range(KD):
                    nc.tensor.matmul(psq[:, ts(dt, 32)],
                                     lhsT=wqpb[:, kd, ts(dt, 128)],
                                     rhs=snT[:, kd, :],
                                     start=(kd == 0), stop=(kd == KD - 1))
            for dt in range(KD):
                nc.scalar.activation(out=qT[:, dt, :], in_=psq[:, ts(dt, 32)],
                                     func=AF.Identity, bias=bqp_col[:, dt, :],
                                     scale=1.0)

            upd_sbs = []
            for bl in range(ROUND):
                # ---- dots^T [N, 8] via k^T-chunk-stationary matmuls ----
                dps = ps.tile([128, NT, S], F32, tag="big")
                for t in range(NT):
                    for kd in range(KD):
                        nc.tensor.matmul(dps[:, t, :],
                                         lhsT=kTs[bl][:, kd, ts(t, 128)],
                                         rhs=qT[:, kd, bl * S:bl * S + S],
                                         start=(kd == 0), stop=(kd == KD - 1))
                # ---- softmax over slots (free dim), fold 1/sqrt(D) ----
                et = itp.tile([128, NT, S], BF16, tag="et")
                nc.scalar.activation(out=et, in_=dps, func=AF.Exp, bias=0.0,
                                     scale=SCALE)
                sig = itp.tile([128, NT, 1], F32, tag="sig")
                nc.vector.tensor_reduce(out=sig[:, :, 0], in_=et, axis=mybir.AxisListType.X,
                                        op=ALU.add)
                nc.vector.reciprocal(out=sig, in_=sig)
                nc.vector.tensor_tensor(out=et, in0=et,
                                        in1=sig.to_broadcast([128, NT, S]),
                                        op=ALU.mult)
                # ---- updates[8, 257] = E^T.T @ v_aug (ones col = denom) ----
                upd = ps.tile([S, D + 1], F32, tag="small")
                for t in range(NT):
                    nc.tensor.matmul(upd, lhsT=et[:, t, :], rhs=vs[bl][:, t, :],
                                     start=(t == 0), stop=(t == NT - 1))
                den = itp.tile([S, 1], F32, tag="den")
                nc.vector.reciprocal(out=den, in_=upd[:, D:D + 1])
                upd_sb = sp.tile([S, D], F32, tag="upd_sb%d" % bl)
                nc.vector.tensor_scalar(out=upd_sb, in0=upd[:, 0:D], scalar1=den,
                                        scalar2=None, op0=ALU.mult)
                upd_sbs.append(upd_sb)

            # ---- batched GRU ----
            updT = sp.tile([128, KD, 32], BF16, tag="updT")
            for bl in range(ROUND):
                pe_t(updT, upd_sbs[bl], S, D, id_f, col0=bl * S)
            slT = sp.tile([128, KD, 32], BF16, tag="slT")
            pe_t(slT, slots_r, 32, D, id_f)
            gx = ps.tile([32, 3 * D], F32, tag="gates")
            gh = ps.tile([32, 3 * D], F32, tag="gates")
            for blk, w in ((0, 512), (512, 256)):
                for kd in range(KD):
                    nc.tensor.matmul(gx[:, blk:blk + w], lhsT=updT[:, kd, :],
                                     rhs=wihb[:, kd, blk:blk + w],
                                     start=(kd == 0), stop=False)
                nc.tensor.matmul(gx[:, blk:blk + w], lhsT=ones_row[:, 0:32],
                                 rhs=gxbias_row[:, blk:blk + w],
                                 start=False, stop=True)
                for kd in range(KD):
                    nc.tensor.matmul(gh[:, blk:blk + w], lhsT=slT[:, kd, :],
                                     rhs=whhb[:, kd, blk:blk + w],
                                     start=(kd == 0), stop=False)
                nc.tensor.matmul(gh[:, blk:blk + w], lhsT=ones_row[:, 0:32],
                                 rhs=ghbias_row[:, blk:blk + w],
                                 start=False, stop=True)
            ghs = sp.tile([32, 3 * D], F32, tag="ghs")
            nc.vector.tensor_copy(out=ghs, in_=gh)
            rz = sp.tile([32, 2 * D], F32, tag="rz")
            nc.vector.tensor_tensor(out=rz, in0=gx[:, 0:2 * D], in1=ghs[:, 0:2 * D],
                                    op=ALU.add)
            nc.scalar.activation(out=rz, in_=rz, func=AF.Sigmoid)
            nsb = sp.tile([32, D], F32, tag="nsb")
            nc.vector.tensor_tensor(out=nsb, in0=rz[:, 0:D], in1=ghs[:, 2 * D:3 * D],
                                    op=ALU.mult)
            nc.vector.tensor_tensor(out=nsb, in0=nsb, in1=gx[:, 2 * D:3 * D],
                                    op=ALU.add)
            nc.scalar.activation(out=nsb, in_=nsb, func=AF.Tanh)
            dlt = sp.tile([32, D], F32, tag="dlt")
            sl = slots_r
            nc.vector.tensor_tensor(out=dlt, in0=sl, in1=nsb, op=ALU.subtract)
            nc.vector.tensor_tensor(out=dlt, in0=dlt, in1=rz[:, D:2 * D], op=ALU.mult)
            nc.vector.tensor_tensor(out=sl, in0=nsb, in1=dlt, op=ALU.add)

            # ---- batched MLP with pre-LN ----
            ffb = sp.tile([32, D], BF16, tag="ffb")
            slot_ln_cast(32, ffb, slots_r)
            ffT = sp.tile([128, KD, 32], BF16, tag="ffT")
            pe_t(ffT, ffb, 32, D, id_b)
            h1 = ps.tile([32, H], F32, tag="gates")
            for blk in (0, 512):
                for kd in range(KD):
                    nc.tensor.matmul(h1[:, blk:blk + 512], lhsT=ffT[:, kd, :],
                                     rhs=w1pb[:, kd, blk:blk + 512],
                                     start=(kd == 0), stop=False)
                nc.tensor.matmul(h1[:, blk:blk + 512], lhsT=ones_row[:, 0:32],
                                 rhs=b1p_row[:, blk:blk + 512],
                                 start=False, stop=True)
            h1b = sp.tile([32, H], BF16, tag="h1b")
            nc.scalar.activation(out=h1b, in_=h1, func=AF.Relu)
            h1T = sp.tile([128, KH, 32], BF16, tag="h1T")
            pe_t(h1T, h1b, 32, H, id_b)
            h2 = ps.tile([32, D], F32, tag="small")
            for kh in range(KH):
                nc.tensor.matmul(h2, lhsT=h1T[:, kh, :], rhs=w2b[:, kh, :],
                                 start=(kh == 0), stop=False)
            nc.tensor.matmul(h2, lhsT=ones_row[:, 0:32], rhs=b2_row,
                             start=False, stop=True)
            nc.vector.tensor_tensor(out=sl, in0=sl, in1=h2, op=ALU.add)

        nc.sync.dma_start(
            out=out_dram[:, rnd * ROUND:(rnd + 1) * ROUND, :].rearrange(
                "s b d -> b s d"),
            in_=slots_r)




_NC_CACHE = None


def _get_nc():
    global _NC_CACHE
    if _NC_CACHE is None:
        _NC_CACHE = _build()
    return _NC_CACHE


def kernel(inputs, slots, **w):
    inputs = np.ascontiguousarray(np.asarray(inputs, np.float32))
    slots = np.ascontiguousarray(np.asarray(slots, np.float32))
    B = inputs.shape[0]
    n_cores = 8
    bs = B // n_cores
    nc = _get_nc()
    wmaps = {k: np.ascontiguousarray(np.asarray(v, np.float32)) for k, v in w.items()}
    in_maps = []
    for c in range(n_cores):
        m = dict(wmaps)
        m["inputs"] = inputs[c * bs:(c + 1) * bs]
        m["slots"] = np.ascontiguousarray(slots[:, c * bs:(c + 1) * bs, :])
        in_maps.append(m)
    res = run_bass_kernel_spmd(nc, in_maps, core_ids=list(range(n_cores)))
    out = np.concatenate([r["out"] for r in res.results], axis=1)
    return np.ascontiguousarray(out.astype(np.float32))


if __name__ == "__main__":
    nc = _build()
    print("built ok; instructions:", len(nc.inst_map))


# revision 20
# speedup vs baseline: 1.6358x; 1.6358x over previous
"""Slot-attention module Bass/Tile kernel (nn_AttentionModule_39084202394083).

Contract: kernel(**inputs) takes FULL unsharded inputs (B=64, N=4096, D=256,
S=8 slots, 3 iterations) and returns the FULL output [S=8, B=64, D=256] f32.

Sharding: data-parallel over batch B across 8 NeuronCores (8 batch elements
per core, all params replicated); no collectives.

Design (v2 — reassociated attention, no k/v materialization):
  - All LN affine params and projection chains are folded on the HOST:
      dots   = x^T.T @ (wkp @ q^T)        (k never materialized)
      gru_x  = ((E@x_hat)/den) @ (wvp @ w_ih) + bias   (v never materialized)
    so the only big on-device tensors are x_hat (LN'd input, natural layout,
    with a ones column -> attention denominators fall out of the E@x matmul)
    and x^T (for the dots stationary operand), both bf16.
  - dots are computed transposed [N, 8] so softmax over slots is a free-dim
    reduce; exp folds the 1/sqrt(D) scale; sigma is applied to E^T via a
    free-dim-broadcast tensor_tensor multiply.
  - Slot state lives at 32-spaced partition rows (row = 32*b_in_round + s),
    so the E@x matmuls for 4 batches col-tile into PSUM partition groups via
    tile_position, and all slot-side GEMMs/element-wise ops run batched on
    [128, *] tiles with junk lanes zeroed.
  - SBUF->SBUF DMA xbar transposes build x^T; they alternate between the two
    HWDGE engines (sync + scalar) to halve the serial dispatch cost.
  - Batches run in 2 rounds of 4 so bf16 x_hat/x^T for a round fits SBUF.
"""

import numpy as np
import ml_dtypes
from contextlib import ExitStack

import concourse.bass as bass
import concourse.tile as tile
from concourse import bacc, mybir
from concourse.bass import ts
from concourse.bass_utils import run_bass_kernel_spmd
from concourse.masks import make_identity

F32 = mybir.dt.float32
BF16 = mybir.dt.bfloat16
AF = mybir.ActivationFunctionType
ALU = mybir.AluOpType

B_LOC = 8          # batch elements per core
N = 4096           # positions
D = 256            # model dim
S = 8              # slots
H = 1024           # mlp hidden
NT = N // 128      # 32 position tiles
KD = 2             # 128-chunks of D
KH = H // 128      # 8 128-chunks of H
ITERS = 3
ROUND = 4          # batches per round
NROUND = B_LOC // ROUND
SUP = 4            # position tiles per load super-tile
LN_EPS = 1e-5
SCALE = float(D) ** -0.5

BF = ml_dtypes.bfloat16

# folded-weight dram tensors: name -> (shape, dtype)
FW_NAMES = {
    "wkpT": ([D, D], BF16),      # (diag(g_in) wk)^T
    "wqp": ([D, D], BF16),       # diag(g_sl) wq
    "wvih": ([D, 3 * D], BF16),  # (diag(g_in) wv) @ w_ih
    "whh": ([D, 3 * D], BF16),
    "w1p": ([D, H], BF16),       # diag(g_ff) mlp_w1
    "w2": ([H, D], BF16),
    "gxbias": ([3 * D], BF16),   # (b_in@wv + bv)@w_ih + b_ih
    "ghbias": ([3 * D], BF16),   # b_hh
}
# optional (only shipped when nonzero): bkp [D], bqp [D], b1p [H], b2 [D]


def _build(flags):
    has_kbias, has_qbias, has_b1, has_b2 = flags
    nc = bacc.Bacc("TRN2", debug=False, enable_asserts=False)
    inp = nc.dram_tensor("inputs", [B_LOC, N, D], F32, kind="ExternalInput").ap()
    slots_in = nc.dram_tensor("slots", [S, B_LOC, D], F32, kind="ExternalInput").ap()
    W = {}
    for name, (shape, dt_) in FW_NAMES.items():
        W[name] = nc.dram_tensor(name, shape, dt_, kind="ExternalInput").ap()
    if has_kbias:
        W["bkp"] = nc.dram_tensor("bkp", [D], F32, kind="ExternalInput").ap()
    if has_qbias:
        W["bqp"] = nc.dram_tensor("bqp", [D], F32, kind="ExternalInput").ap()
    if has_b1:
        W["b1p"] = nc.dram_tensor("b1p", [H], F32, kind="ExternalInput").ap()
    if has_b2:
        W["b2"] = nc.dram_tensor("b2", [D], BF16, kind="ExternalInput").ap()
    out_dram = nc.dram_tensor("out", [S, B_LOC, D], F32, kind="ExternalOutput").ap()

    with tile.TileContext(nc) as tc:
        with ExitStack() as ctx:
            _body(ctx, tc, inp, slots_in, W, out_dram, flags)
    nc.compile()
    return nc


def _body(ctx, tc, inp, slots_in, W, out_dram, flags):
    has_kbias, has_qbias, has_b1, has_b2 = flags
    nc = tc.nc

    wts = ctx.enter_context(tc.tile_pool(name="wts", bufs=1))
    ps = ctx.enter_context(tc.tile_pool(name="ps", bufs=2, space="PSUM"))
    psg = ctx.enter_context(tc.tile_pool(name="psg", bufs=1, space="PSUM"))

    # ---------------- constants / weights (host-folded) ----------------
    id_f = wts.tile([128, 128], F32, tag="idf")
    make_identity(nc, id_f)
    id_b = wts.tile([128, 128], BF16, tag="idb")
    make_identity(nc, id_b)
    ones_row = wts.tile([1, 128], BF16, tag="ones_row")
    nc.vector.memset(ones_row, 1.0)
    eps_col = wts.tile([128, 1], F32, tag="eps_col")
    nc.vector.memset(eps_col, LN_EPS)

    def wload(name, kchunks, cols, dt_=BF16):
        t = wts.tile([128, kchunks, cols], dt_, tag="w_" + name)
        nc.sync.dma_start(out=t, in_=W[name].rearrange("(k p) c -> p k c", p=128))
        return t

    def wrow(name, n):
        t = wts.tile([1, n], BF16, tag="w_" + name)
        nc.sync.dma_start(out=t, in_=W[name].rearrange("(one a) -> one a", one=1))
        return t

    wkpT_b = wload("wkpT", KD, D)
    wqp_b = wload("wqp", KD, D)
    wvih_b = wload("wvih", KD, 3 * D)
    whh_b = wload("whh", KD, 3 * D)
    w1p_b = wload("w1p", KD, H)
    w2_b = wload("w2", KH, D)
    gxbias_row = wrow("gxbias", 3 * D)
    ghbias_row = wrow("ghbias", 3 * D)
    bkp_col = wload("bkp", KD, 1, F32) if has_kbias else None
    bqp_col = wload("bqp", KD, 1, F32) if has_qbias else None
    b1p_col = wload("b1p", KH, 1, F32) if has_b1 else None
    b2_row = wrow("b2", D) if has_b2 else None

    kvx = ctx.enter_context(tc.tile_pool(name="kvx", bufs=ROUND))
    slot_pool = ctx.enter_context(tc.tile_pool(name="slot", bufs=NROUND))
    xload = ctx.enter_context(tc.tile_pool(name="xload", bufs=2))
    stat = ctx.enter_context(tc.tile_pool(name="stat", bufs=2))
    itp = ctx.enter_context(tc.tile_pool(name="itp", bufs=2))
    sp = ctx.enter_context(tc.tile_pool(name="sp", bufs=1))

    dma_t_engines = [nc.sync, nc.scalar]
    dma_t_idx = [0]

    def phase_a(b):
        """Load + LN + cast: returns (x_aug [128,NT,257] bf16, xT [128,KD,N] bf16)."""
        x_aug = kvx.tile([128, NT, D + 1], BF16, tag="xa")
        nc.vector.memset(x_aug[:, :, D:D + 1], 1.0)
        xT = kvx.tile([128, KD, N], BF16, tag="xT")
        inp_b = inp[b].rearrange("(c p) d -> p c d", p=128)
        mv_all = stat.tile([128, NT, 2], F32, tag="mv")
        r_all = stat.tile([128, NT, 1], F32, tag="r")

        for g in range(NT // SUP):
            xs = xload.tile([128, SUP, D], F32, tag="xs")
            nc.sync.dma_start(out=xs, in_=inp_b[:, ts(g, SUP), :])
            st6 = stat.tile([128, SUP, 6], F32, tag="st6")
            for j in range(SUP):
                nc.vector.bn_stats(out=st6[:, j, :], in_=xs[:, j, :])
            for j in range(SUP):
                nc.vector.bn_aggr(out=mv_all[:, g * SUP + j, :], in_=st6[:, j, :])
            nc.scalar.activation(out=r_all[:, ts(g, SUP), :],
                                 in_=mv_all[:, ts(g, SUP), 1:2],
                                 func=AF.Sqrt, bias=eps_col, scale=1.0)
            nc.vector.reciprocal(out=r_all[:, ts(g, SUP), :],
                                 in_=r_all[:, ts(g, SUP), :])
            for j in range(SUP):
                t = g * SUP + j
                nc.vector.tensor_scalar(out=x_aug[:, t, 0:D], in0=xs[:, j, :],
                                        scalar1=mv_all[:, t, 0:1],
                                        scalar2=r_all[:, t, 0:1],
                                        op0=ALU.subtract, op1=ALU.mult)
                for kd in range(KD):
                    eng = dma_t_engines[dma_t_idx[0] % 2]
                    dma_t_idx[0] += 1
                    eng.dma_start_transpose(out=xT[:, kd, ts(t, 128)],
                                            in_=x_aug[:, t, ts(kd, 128)])
        return x_aug, xT

    def pe_t(dst, src, identity):
        """dst[128, KD, 128] (bf16) = src[128, 256].T via PE transpose."""
        dt_ = src.dtype
        for j in range(KD):
            pst = ps.tile([128, 128], dt_, tag="small")
            nc.tensor.transpose(out=pst, in_=src[:, ts(j, 128)], identity=identity)
            nc.vector.tensor_copy(out=dst[:, j, :], in_=pst)

    def slot_ln_cast(sl, out_bf):
        st6 = sp.tile([128, 6], F32, tag="sst6")
        mv = sp.tile([128, 2], F32, tag="smv")
        rr = sp.tile([128, 1], F32, tag="srr")
        nc.vector.bn_stats(out=st6, in_=sl)
        nc.vector.bn_aggr(out=mv, in_=st6)
        nc.scalar.activation(out=rr, in_=mv[:, 1:2], func=AF.Sqrt,
                             bias=eps_col, scale=1.0)
        nc.vector.reciprocal(out=rr, in_=rr)
        nc.vector.tensor_scalar(out=out_bf, in0=sl, scalar1=mv[:, 0:1],
                                scalar2=rr, op0=ALU.subtract, op1=ALU.mult)

    for rnd in range(NROUND):
        slots_r = slot_pool.tile([128, D], F32, tag="slots")
        nc.vector.memset(slots_r, 0.0)
        for bl in range(ROUND):
            nc.sync.dma_start(out=slots_r[32 * bl:32 * bl + S, :],
                              in_=slots_in[:, rnd * ROUND + bl, :])
        xas, xTs = [], []
        for bl in range(ROUND):
            xa, xT = phase_a(rnd * ROUND + bl)
            xas.append(xa)
            xTs.append(xT)

        for it in range(ITERS):
            # ---- q^T then wqk = wkp @ q^T  (both [128, KD, 128] bf16) ----
            sn = sp.tile([128, D], BF16, tag="sn")
            slot_ln_cast(slots_r, sn)
            snT = sp.tile([128, KD, 128], BF16, tag="snT")
            pe_t(snT, sn, id_b)
            qT = sp.tile([128, KD, 128], BF16, tag="qT")
            for dt in range(KD):
                psq = ps.tile([128, 128], F32, tag="small")
                for kd in range(KD):
                    nc.tensor.matmul(psq, lhsT=wqp_b[:, kd, ts(dt, 128)],
                                     rhs=snT[:, kd, :],
                                     start=(kd == 0), stop=(kd == KD - 1))
                if has_qbias:
                    nc.scalar.activation(out=qT[:, dt, :], in_=psq,
                                         func=AF.Identity,
                                         bias=bqp_col[:, dt, :], scale=1.0)
                else:
                    nc.scalar.copy(out=qT[:, dt, :], in_=psq)
            wqk = sp.tile([128, KD, 128], BF16, tag="wqk")
            for dt in range(KD):
                pwq = ps.tile([128, 128], F32, tag="small")
                for kd in range(KD):
                    nc.tensor.matmul(pwq, lhsT=wkpT_b[:, kd, ts(dt, 128)],
                                     rhs=qT[:, kd, :],
                                     start=(kd == 0), stop=(kd == KD - 1))
                nc.vector.tensor_copy(out=wqk[:, dt, :], in_=pwq)
            if has_kbias:
                pqb = ps.tile([1, 128], F32, tag="small")
                for kd in range(KD):
                    nc.tensor.matmul(pqb, lhsT=bkp_col[:, kd, :],
                                     rhs=qT[:, kd, :],
                                     start=(kd == 0), stop=(kd == KD - 1))
                qb_row = sp.tile([1, 128], BF16, tag="qb_row")
                nc.vector.tensor_copy(out=qb_row, in_=pqb)

            # ---- per batch: dots^T -> softmax -> E^T ----
            ets = []
            for bl in range(ROUND):
                dps = ps.tile([128, NT, S], F32, tag="dps")
                for t in range(NT):
                    for kd in range(KD):
                        nc.tensor.matmul(dps[:, t, :],
                                         lhsT=xTs[bl][:, kd, ts(t, 128)],
                                         rhs=wqk[:, kd, 32 * bl:32 * bl + S],
                                         start=(kd == 0),
                                         stop=(kd == KD - 1 and not has_kbias))
                    if has_kbias:
                        nc.tensor.matmul(dps[:, t, :], lhsT=ones_row,
                                         rhs=qb_row[:, 32 * bl:32 * bl + S],
                                         start=False, stop=True)
                et = itp.tile([128, NT, S], BF16, tag="et%d" % bl)
                nc.scalar.activation(out=et, in_=dps, func=AF.Exp, bias=0.0,
                                     scale=SCALE)
                sig = itp.tile([128, NT, 1], F32, tag="sig")
                nc.vector.tensor_reduce(out=sig, in_=et,
                                        axis=mybir.AxisListType.X, op=ALU.add)
                nc.vector.reciprocal(out=sig, in_=sig)
                nc.vector.tensor_tensor(out=et, in0=et,
                                        in1=sig.to_broadcast([128, NT, S]),
                                        op=ALU.mult)
                ets.append(et)

            # ---- eu = E^T.T @ x_aug, col-tiled 2 batches per PSUM bank ----
            eu_sb = sp.tile([128, D], BF16, tag="eu_sb")
            nc.vector.memset(eu_sb, 0.0)
            den_t = sp.tile([128, 1], F32, tag="den_t")
            for pair in range(ROUND // 2):
                eup = ps.tile([128, D + 1], F32, tag="eu")
                for t in range(NT):
                    for g in range(2):
                        bl = 2 * pair + g
                        r0 = 32 * bl
                        nc.tensor.matmul(eup[r0:r0 + S, :],
                                         lhsT=ets[bl][:, t, :],
                                         rhs=xas[bl][:, t, :],
                                         start=(t == 0), stop=(t == NT - 1),
                                         tile_position=(0, r0),
                                         skip_group_check=True)
                for g in range(2):
                    r0 = 32 * (2 * pair + g)
                    nc.vector.reciprocal(out=den_t[r0:r0 + S, :],
                                         in_=eup[r0:r0 + S, D:D + 1])
                    nc.vector.tensor_scalar(out=eu_sb[r0:r0 + S, :],
                                            in0=eup[r0:r0 + S, 0:D],
                                            scalar1=den_t[r0:r0 + S, :],
                                            scalar2=None, op0=ALU.mult)
            euT = sp.tile([128, KD, 128], BF16, tag="euT")
            pe_t(euT, eu_sb, id_b)

            # ---- batched GRU (rows 32*bl + s) ----
            gx = psg.tile([128, 3 * D], F32, tag="gates")
            for blk, w in ((0, 512), (512, 256)):
                for kd in range(KD):
                    nc.tensor.matmul(gx[:, blk:blk + w], lhsT=euT[:, kd, :],
                                     rhs=wvih_b[:, kd, blk:blk + w],
                                     start=(kd == 0), stop=False)
                nc.tensor.matmul(gx[:, blk:blk + w], lhsT=ones_row,
                                 rhs=gxbias_row[:, blk:blk + w],
                                 start=False, stop=True)
            gxs = sp.tile([128, 3 * D], F32, tag="gxs")
            nc.vector.tensor_copy(out=gxs, in_=gx)
            slT = sp.tile([128, KD, 128], BF16, tag="slT")
            pe_t(slT, slots_r, id_f)
            gh = psg.tile([128, 3 * D], F32, tag="gates")
            for blk, w in ((0, 512), (512, 256)):
                for kd in range(KD):
                    nc.tensor.matmul(gh[:, blk:blk + w], lhsT=slT[:, kd, :],
                                     rhs=whh_b[:, kd, blk:blk + w],
                                     start=(kd == 0), stop=False)
                nc.tensor.matmul(gh[:, blk:blk + w], lhsT=ones_row,
                                 rhs=ghbias_row[:, blk:blk + w],
                                 start=False, stop=True)
            rz = sp.tile([128, 2 * D], F32, tag="rz")
            nc.vector.tensor_tensor(out=rz, in0=gxs[:, 0:2 * D],
                                    in1=gh[:, 0:2 * D], op=ALU.add)
            nc.scalar.activation(out=rz, in_=rz, func=AF.Sigmoid)
            nsb = sp.tile([128, D], F32, tag="nsb")
            nc.vector.tensor_tensor(out=nsb, in0=rz[:, 0:D],
                                    in1=gh[:, 2 * D:3 * D], op=ALU.mult)
            nc.vector.tensor_tensor(out=nsb, in0=nsb, in1=gxs[:, 2 * D:3 * D],
                                    op=ALU.add)
            nc.scalar.activation(out=nsb, in_=nsb, func=AF.Tanh)
            dlt = sp.tile([128, D], F32, tag="dlt")
            nc.vector.tensor_tensor(out=dlt, in0=slots_r, in1=nsb,
                                    op=ALU.subtract)
            nc.vector.tensor_tensor(out=dlt, in0=dlt, in1=rz[:, D:2 * D],
                                    op=ALU.mult)
            nc.vector.tensor_tensor(out=slots_r, in0=nsb, in1=dlt, op=ALU.add)

            # ---- batched MLP with pre-LN; h1 produced transposed ----
            ffb = sp.tile([128, D], BF16, tag="ffb")
            slot_ln_cast(slots_r, ffb)
            ffT = sp.tile([128, KD, 128], BF16, tag="ffT")
            pe_t(ffT, ffb, id_b)
            h1T = sp.tile([128, KH, 128], BF16, tag="h1T")
            for ht in range(KH):
                psh = ps.tile([128, 128], F32, tag="small")
                for kd in range(KD):
                    nc.tensor.matmul(psh, lhsT=w1p_b[:, kd, ts(ht, 128)],
                                     rhs=ffT[:, kd, :],
                                     start=(kd == 0), stop=(kd == KD - 1))
                if has_b1:
                    nc.scalar.activation(out=h1T[:, ht, :], in_=psh,
                                         func=AF.Relu,
                                         bias=b1p_col[:, ht, :], scale=1.0)
                else:
                    nc.scalar.activation(out=h1T[:, ht, :], in_=psh,
                                         func=AF.Relu)
            ps2 = ps.tile([128, D], F32, tag="small")
            for kh in range(KH):
                nc.tensor.matmul(ps2, lhsT=h1T[:, kh, :], rhs=w2_b[:, kh, :],
                                 start=(kh == 0),
                                 stop=(kh == KH - 1 and not has_b2))
            if has_b2:
                nc.tensor.matmul(ps2, lhsT=ones_row, rhs=b2_row,
                                 start=False, stop=True)
            nc.vector.tensor_tensor(out=slots_r, in0=slots_r, in1=ps2,
                                    op=ALU.add)

        for bl in range(ROUND):
            nc.sync.dma_start(out=out_dram[:, rnd * ROUND + bl, :],
                              in_=slots_r[32 * bl:32 * bl + S, :])


def host_fold(w):
    """Fold LN affine params + projection chains on the host (numpy, fp32)."""
    g_in, b_in = w["g_in"], w["b_in"]
    wkp = g_in[:, None] * w["wk"]
    wvp = g_in[:, None] * w["wv"]
    fw = {
        "wkpT": np.ascontiguousarray(wkp.T),
        "wqp": w["g_sl"][:, None] * w["wq"],
        "wvih": wvp @ w["w_ih"],
        "whh": w["w_hh"],
        "w1p": w["g_ff"][:, None] * w["mlp_w1"],
        "w2": w["mlp_w2"],
        "gxbias": (b_in @ w["wv"] + w["bv"]) @ w["w_ih"] + w["b_ih"],
        "ghbias": w["b_hh"],
    }
    fw = {k: np.ascontiguousarray(v).astype(BF) for k, v in fw.items()}
    bkp = (b_in @ w["wk"] + w["bk"]).astype(np.float32)
    bqp = (w["b_sl"] @ w["wq"] + w["bq"]).astype(np.float32)
    b1p = (w["b_ff"] @ w["mlp_w1"] + w["mlp_b1"]).astype(np.float32)
    b2 = w["mlp_b2"].astype(np.float32)
    flags = (bool(np.any(bkp)), bool(np.any(bqp)), bool(np.any(b1p)),
             bool(np.any(b2)))
    if flags[0]:
        fw["bkp"] = bkp
    if flags[1]:
        fw["bqp"] = bqp
    if flags[2]:
        fw["b1p"] = b1p
    if flags[3]:
        fw["b2"] = b2.astype(BF)
    return fw, flags


_NC_CACHE = {}


def get_nc(flags):
    if flags not in _NC_CACHE:
        _NC_CACHE[flags] = _build(flags)
    return _NC_CACHE[flags]


def prepare(inputs, slots, **w):
    """Returns (nc, in_maps) for the 8-core SPMD launch."""
    inputs = np.ascontiguousarray(np.asarray(inputs, np.float32))
    slots = np.ascontiguousarray(np.asarray(slots, np.float32))
    w = {k: np.asarray(v, np.float32) for k, v in w.items()}
    fw, flags = host_fold(w)
    nc = get_nc(flags)
    n_cores = 8
    bs = inputs.shape[0] // n_cores
    in_maps = []
    for c in range(n_cores):
        m = dict(fw)
        m["inputs"] = inputs[c * bs:(c + 1) * bs]
        m["slots"] = np.ascontiguousarray(slots[:, c * bs:(c + 1) * bs, :])
        in_maps.append(m)
    return nc, in_maps


def kernel(inputs, slots, **w):
    nc, in_maps = prepare(inputs, slots, **w)
    res = run_bass_kernel_spmd(nc, in_maps, core_ids=list(range(len(in_maps))))
    out = np.concatenate([r["out"] for r in res.results], axis=1)
    return np.ascontiguousarray(out.astype(np.float32))


if __name__ == "__main__":
    nc = _build((False, False, False, False))
    print("built ok; instructions:", len(nc.inst_map))


# revision 21
# speedup vs baseline: 2.4787x; 1.5153x over previous
"""Slot-attention module Bass/Tile kernel (nn_AttentionModule_39084202394083).

Contract: kernel(**inputs) takes FULL unsharded inputs (B=64, N=4096, D=256,
S=8 slots, 3 iterations) and returns the FULL output [S=8, B=64, D=256] f32.

Sharding: data-parallel over batch B across 8 NeuronCores (8 batch elements
per core, all params replicated); no collectives.

Design (v2 — reassociated attention, no k/v materialization):
  - All LN affine params and projection chains are folded on the HOST:
      dots   = x^T.T @ (wkp @ q^T)        (k never materialized)
      gru_x  = ((E@x_hat)/den) @ (wvp @ w_ih) + bias   (v never materialized)
    so the only big on-device tensors are x_hat (LN'd input, natural layout,
    with a ones column -> attention denominators fall out of the E@x matmul)
    and x^T (for the dots stationary operand), both bf16.
  - dots are computed transposed [N, 8] so softmax over slots is a free-dim
    reduce; exp folds the 1/sqrt(D) scale; sigma is applied to E^T via a
    free-dim-broadcast tensor_tensor multiply.
  - Slot state lives at 32-spaced partition rows (row = 32*b_in_round + s),
    so the E@x matmuls for 4 batches col-tile into PSUM partition groups via
    tile_position, and all slot-side GEMMs/element-wise ops run batched on
    [128, *] tiles with junk lanes zeroed.
  - SBUF->SBUF DMA xbar transposes build x^T; they alternate between the two
    HWDGE engines (sync + scalar) to halve the serial dispatch cost.
  - Batches run in 2 rounds of 4 so bf16 x_hat/x^T for a round fits SBUF.
"""

import numpy as np
import ml_dtypes
from contextlib import ExitStack

import concourse.bass as bass
import concourse.tile as tile
from concourse import bacc, mybir
from concourse.bass import ts
from concourse.bass_utils import run_bass_kernel_spmd
from concourse.masks import make_identity

F32 = mybir.dt.float32
BF16 = mybir.dt.bfloat16
AF = mybir.ActivationFunctionType
ALU = mybir.AluOpType

B_LOC = 8          # batch elements per core
N = 4096           # positions
D = 256            # model dim
S = 8              # slots
H = 1024           # mlp hidden
NT = N // 128      # 32 position tiles
KD = 2             # 128-chunks of D
KH = H // 128      # 8 128-chunks of H
ITERS = 3
ROUND = 4          # batches per round
NROUND = B_LOC // ROUND
SUP = 4            # position tiles per load super-tile
LN_EPS = 1e-5
SCALE = float(D) ** -0.5

BF = ml_dtypes.bfloat16

# folded-weight dram tensors: name -> (shape, dtype)
FW_NAMES = {
    "wkpT": ([D, D], BF16),      # (diag(g_in) wk)^T
    "wqp": ([D, D], BF16),       # diag(g_sl) wq
    "wvih": ([D, 3 * D], BF16),  # (diag(g_in) wv) @ w_ih
    "whh": ([D, 3 * D], BF16),
    "w1p": ([D, H], BF16),       # diag(g_ff) mlp_w1
    "w2": ([H, D], BF16),
    "gxbias": ([3 * D], BF16),   # (b_in@wv + bv)@w_ih + b_ih
    "ghbias": ([3 * D], BF16),   # b_hh
}
# optional (only shipped when nonzero): bkp [D], bqp [D], b1p [H], b2 [D]


def _build(flags):
    has_kbias, has_qbias, has_b1, has_b2 = flags
    nc = bacc.Bacc("TRN2", debug=False, enable_asserts=False)
    inp = nc.dram_tensor("inputs", [B_LOC, N, D], F32, kind="ExternalInput").ap()
    slots_in = nc.dram_tensor("slots", [S, B_LOC, D], F32, kind="ExternalInput").ap()
    W = {}
    for name, (shape, dt_) in FW_NAMES.items():
        W[name] = nc.dram_tensor(name, shape, dt_, kind="ExternalInput").ap()
    if has_kbias:
        W["bkp"] = nc.dram_tensor("bkp", [D], F32, kind="ExternalInput").ap()
    if has_qbias:
        W["bqp"] = nc.dram_tensor("bqp", [D], F32, kind="ExternalInput").ap()
    if has_b1:
        W["b1p"] = nc.dram_tensor("b1p", [H], F32, kind="ExternalInput").ap()
    if has_b2:
        W["b2"] = nc.dram_tensor("b2", [D], BF16, kind="ExternalInput").ap()
    out_dram = nc.dram_tensor("out", [S, B_LOC, D], F32, kind="ExternalOutput").ap()

    with tile.TileContext(nc) as tc:
        with ExitStack() as ctx:
            _body(ctx, tc, inp, slots_in, W, out_dram, flags)
    nc.compile()
    return nc


def _body(ctx, tc, inp, slots_in, W, out_dram, flags):
    has_kbias, has_qbias, has_b1, has_b2 = flags
    nc = tc.nc

    wts = ctx.enter_context(tc.tile_pool(name="wts", bufs=1))
    ps = ctx.enter_context(tc.tile_pool(name="ps", bufs=2, space="PSUM"))
    psg = ctx.enter_context(tc.tile_pool(name="psg", bufs=1, space="PSUM"))

    # ---------------- constants / weights (host-folded) ----------------
    id_f = wts.tile([128, 128], F32, tag="idf")
    make_identity(nc, id_f)
    id_b = wts.tile([128, 128], BF16, tag="idb")
    make_identity(nc, id_b)
    ones_row = wts.tile([1, 128], BF16, tag="ones_row")
    nc.vector.memset(ones_row, 1.0)
    eps_col = wts.tile([128, 1], F32, tag="eps_col")
    nc.vector.memset(eps_col, LN_EPS)

    def wload(name, kchunks, cols, dt_=BF16):
        t = wts.tile([128, kchunks, cols], dt_, tag="w_" + name)
        nc.sync.dma_start(out=t, in_=W[name].rearrange("(k p) c -> p k c", p=128))
        return t

    def wrow(name, n):
        t = wts.tile([1, n], BF16, tag="w_" + name)
        nc.sync.dma_start(out=t, in_=W[name].rearrange("(one a) -> one a", one=1))
        return t

    wkpT_b = wload("wkpT", KD, D)
    wqp_b = wload("wqp", KD, D)
    wvih_b = wload("wvih", KD, 3 * D)
    whh_b = wload("whh", KD, 3 * D)
    w1p_b = wload("w1p", KD, H)
    w2_b = wload("w2", KH, D)
    gxbias_row = wrow("gxbias", 3 * D)
    ghbias_row = wrow("ghbias", 3 * D)
    bkp_col = wload("bkp", KD, 1, F32) if has_kbias else None
    bqp_col = wload("bqp", KD, 1, F32) if has_qbias else None
    b1p_col = wload("b1p", KH, 1, F32) if has_b1 else None
    b2_row = wrow("b2", D) if has_b2 else None

    kvx = ctx.enter_context(tc.tile_pool(name="kvx", bufs=ROUND))
    slot_pool = ctx.enter_context(tc.tile_pool(name="slot", bufs=NROUND))
    xload = ctx.enter_context(tc.tile_pool(name="xload", bufs=2))
    stat = ctx.enter_context(tc.tile_pool(name="stat", bufs=2))
    itp = ctx.enter_context(tc.tile_pool(name="itp", bufs=2))
    sp = ctx.enter_context(tc.tile_pool(name="sp", bufs=1))

    def phase_a(b):
        """Load + LN + cast: returns (x_aug [128,NT,257] bf16, xT [128,KD,N] bf16)."""
        x_aug = kvx.tile([128, NT, D + 1], BF16, tag="xa")
        nc.vector.memset(x_aug[:, :, D:D + 1], 1.0)
        xT = kvx.tile([128, KD, N], BF16, tag="xT")
        inp_b = inp[b].rearrange("(c p) d -> p c d", p=128)
        mv_all = stat.tile([128, NT, 2], F32, tag="mv")
        r_all = stat.tile([128, NT, 1], F32, tag="r")

        for g in range(NT // SUP):
            xs = xload.tile([128, SUP, D], F32, tag="xs")
            nc.sync.dma_start(out=xs, in_=inp_b[:, ts(g, SUP), :])
            st6 = stat.tile([128, SUP, 6], F32, tag="st6")
            for j in range(SUP):
                nc.vector.bn_stats(out=st6[:, j, :], in_=xs[:, j, :])
            for j in range(SUP):
                nc.vector.bn_aggr(out=mv_all[:, g * SUP + j, :], in_=st6[:, j, :])
            nc.scalar.activation(out=r_all[:, ts(g, SUP), :],
                                 in_=mv_all[:, ts(g, SUP), 1:2],
                                 func=AF.Sqrt, bias=eps_col, scale=1.0)
            nc.vector.reciprocal(out=r_all[:, ts(g, SUP), :],
                                 in_=r_all[:, ts(g, SUP), :])
            for j in range(SUP):
                t = g * SUP + j
                nc.vector.tensor_scalar(out=x_aug[:, t, 0:D], in0=xs[:, j, :],
                                        scalar1=mv_all[:, t, 0:1],
                                        scalar2=r_all[:, t, 0:1],
                                        op0=ALU.subtract, op1=ALU.mult)
                for kd in range(KD):
                    pst = ps.tile([128, 128], BF16, tag="small")
                    nc.tensor.transpose(out=pst, in_=x_aug[:, t, ts(kd, 128)],
                                        identity=id_b)
                    nc.vector.tensor_copy(out=xT[:, kd, ts(t, 128)], in_=pst)
        return x_aug, xT

    def pe_t(dst, src, identity):
        """dst[128, KD, 128] (bf16) = src[128, 256].T via PE transpose."""
        dt_ = src.dtype
        for j in range(KD):
            pst = ps.tile([128, 128], dt_, tag="small")
            nc.tensor.transpose(out=pst, in_=src[:, ts(j, 128)], identity=identity)
            nc.vector.tensor_copy(out=dst[:, j, :], in_=pst)

    def slot_ln_cast(sl, out_bf):
        st6 = sp.tile([128, 6], F32, tag="sst6")
        mv = sp.tile([128, 2], F32, tag="smv")
        rr = sp.tile([128, 1], F32, tag="srr")
        nc.vector.bn_stats(out=st6, in_=sl)
        nc.vector.bn_aggr(out=mv, in_=st6)
        nc.scalar.activation(out=rr, in_=mv[:, 1:2], func=AF.Sqrt,
                             bias=eps_col, scale=1.0)
        nc.vector.reciprocal(out=rr, in_=rr)
        nc.vector.tensor_scalar(out=out_bf, in0=sl, scalar1=mv[:, 0:1],
                                scalar2=rr, op0=ALU.subtract, op1=ALU.mult)

    for rnd in range(NROUND):
        slots_r = slot_pool.tile([128, D], F32, tag="slots")
        nc.vector.memset(slots_r, 0.0)
        for bl in range(ROUND):
            nc.sync.dma_start(out=slots_r[32 * bl:32 * bl + S, :],
                              in_=slots_in[:, rnd * ROUND + bl, :])
        xas, xTs = [], []
        for bl in range(ROUND):
            xa, xT = phase_a(rnd * ROUND + bl)
            xas.append(xa)
            xTs.append(xT)

        for it in range(ITERS):
            # ---- q^T then wqk = wkp @ q^T  (both [128, KD, 128] bf16) ----
            sn = sp.tile([128, D], BF16, tag="sn")
            slot_ln_cast(slots_r, sn)
            snT = sp.tile([128, KD, 128], BF16, tag="snT")
            pe_t(snT, sn, id_b)
            qT = sp.tile([128, KD, 128], BF16, tag="qT")
            for dt in range(KD):
                psq = ps.tile([128, 128], F32, tag="small")
                for kd in range(KD):
                    nc.tensor.matmul(psq, lhsT=wqp_b[:, kd, ts(dt, 128)],
                                     rhs=snT[:, kd, :],
                                     start=(kd == 0), stop=(kd == KD - 1))
                if has_qbias:
                    nc.scalar.activation(out=qT[:, dt, :], in_=psq,
                                         func=AF.Identity,
                                         bias=bqp_col[:, dt, :], scale=1.0)
                else:
                    nc.vector.tensor_copy(out=qT[:, dt, :], in_=psq)
            wqk = sp.tile([128, KD, 128], BF16, tag="wqk")
            for dt in range(KD):
                pwq = ps.tile([128, 128], F32, tag="small")
                for kd in range(KD):
                    nc.tensor.matmul(pwq, lhsT=wkpT_b[:, kd, ts(dt, 128)],
                                     rhs=qT[:, kd, :],
                                     start=(kd == 0), stop=(kd == KD - 1))
                nc.vector.tensor_copy(out=wqk[:, dt, :], in_=pwq)
            if has_kbias:
                pqb = ps.tile([1, 128], F32, tag="small")
                for kd in range(KD):
                    nc.tensor.matmul(pqb, lhsT=bkp_col[:, kd, :],
                                     rhs=qT[:, kd, :],
                                     start=(kd == 0), stop=(kd == KD - 1))
                qb_row = sp.tile([1, 128], BF16, tag="qb_row")
                nc.vector.tensor_copy(out=qb_row, in_=pqb)

            # ---- per batch: dots^T -> softmax -> E^T ----
            ets = []
            for bl in range(ROUND):
                dps = ps.tile([128, NT, S], F32, tag="dps")
                for t in range(NT):
                    for kd in range(KD):
                        nc.tensor.matmul(dps[:, t, :],
                                         lhsT=xTs[bl][:, kd, ts(t, 128)],
                                         rhs=wqk[:, kd, 32 * bl:32 * bl + S],
                                         start=(kd == 0),
                                         stop=(kd == KD - 1 and not has_kbias))
                    if has_kbias:
                        nc.tensor.matmul(dps[:, t, :], lhsT=ones_row,
                                         rhs=qb_row[:, 32 * bl:32 * bl + S],
                                         start=False, stop=True)
                et = itp.tile([128, NT, S], BF16, tag="et%d" % bl)
                nc.scalar.activation(out=et, in_=dps, func=AF.Exp, bias=0.0,
                                     scale=SCALE)
                sig = itp.tile([128, NT, 1], F32, tag="sig")
                nc.vector.tensor_reduce(out=sig, in_=et,
                                        axis=mybir.AxisListType.X, op=ALU.add)
                nc.vector.reciprocal(out=sig, in_=sig)
                nc.vector.tensor_tensor(out=et, in0=et,
                                        in1=sig.to_broadcast([128, NT, S]),
                                        op=ALU.mult)
                ets.append(et)

            # ---- eu = E^T.T @ x_aug, col-tiled 2 batches per PSUM bank ----
            eu_sb = sp.tile([128, D], BF16, tag="eu_sb")
            nc.vector.memset(eu_sb, 0.0)
            den_t = sp.tile([128, 1], F32, tag="den_t")
            for pair in range(ROUND // 2):
                eup = ps.tile([128, D + 1], F32, tag="eu")
                for t in range(NT):
                    for g in range(2):
                        bl = 2 * pair + g
                        r0 = 32 * bl
                        nc.tensor.matmul(eup[r0:r0 + S, :],
                                         lhsT=ets[bl][:, t, :],
                                         rhs=xas[bl][:, t, :],
                                         start=(t == 0), stop=(t == NT - 1),
                                         tile_position=(0, r0),
                                         skip_group_check=True)
                for g in range(2):
                    r0 = 32 * (2 * pair + g)
                    nc.vector.reciprocal(out=den_t[r0:r0 + S, :],
                                         in_=eup[r0:r0 + S, D:D + 1])
                    nc.vector.tensor_scalar(out=eu_sb[r0:r0 + S, :],
                                            in0=eup[r0:r0 + S, 0:D],
                                            scalar1=den_t[r0:r0 + S, :],
                                            scalar2=None, op0=ALU.mult)
            euT = sp.tile([128, KD, 128], BF16, tag="euT")
            pe_t(euT, eu_sb, id_b)

            # ---- batched GRU (rows 32*bl + s) ----
            gx = psg.tile([128, 3 * D], F32, tag="gates")
            for blk, w in ((0, 512), (512, 256)):
                for kd in range(KD):
                    nc.tensor.matmul(gx[:, blk:blk + w], lhsT=euT[:, kd, :],
                                     rhs=wvih_b[:, kd, blk:blk + w],
                                     start=(kd == 0), stop=False)
                nc.tensor.matmul(gx[:, blk:blk + w], lhsT=ones_row,
                                 rhs=gxbias_row[:, blk:blk + w],
                                 start=False, stop=True)
            gxs = sp.tile([128, 3 * D], F32, tag="gxs")
            nc.vector.tensor_copy(out=gxs, in_=gx)
            slT = sp.tile([128, KD, 128], BF16, tag="slT")
            pe_t(slT, slots_r, id_f)
            gh = psg.tile([128, 3 * D], F32, tag="gates")
            for blk, w in ((0, 512), (512, 256)):
                for kd in range(KD):
                    nc.tensor.matmul(gh[:, blk:blk + w], lhsT=slT[:, kd, :],
                                     rhs=whh_b[:, kd, blk:blk + w],
                                     start=(kd == 0), stop=False)
                nc.tensor.matmul(gh[:, blk:blk + w], lhsT=ones_row,
                                 rhs=ghbias_row[:, blk:blk + w],
                                 start=False, stop=True)
            rz = sp.tile([128, 2 * D], F32, tag="rz")
            nc.vector.tensor_tensor(out=rz, in0=gxs[:, 0:2 * D],
                                    in1=gh[:, 0:2 * D], op=ALU.add)
            nc.scalar.activation(out=rz, in_=rz, func=AF.Sigmoid)
            nsb = sp.tile([128, D], F32, tag="nsb")
            nc.vector.tensor_tensor(out=nsb, in0=rz[:, 0:D],
                                    in1=gh[:, 2 * D:3 * D], op=ALU.mult)
            nc.vector.tensor_tensor(out=nsb, in0=nsb, in1=gxs[:, 2 * D:3 * D],
                                    op=ALU.add)
            nc.scalar.activation(out=nsb, in_=nsb, func=AF.Tanh)
            dlt = sp.tile([128, D], F32, tag="dlt")
            nc.vector.tensor_tensor(out=dlt, in0=slots_r, in1=nsb,
                                    op=ALU.subtract)
            nc.vector.tensor_tensor(out=dlt, in0=dlt, in1=rz[:, D:2 * D],
                                    op=ALU.mult)
            nc.vector.tensor_tensor(out=slots_r, in0=nsb, in1=dlt, op=ALU.add)

            # ---- batched MLP with pre-LN; h1 produced transposed ----
            ffb = sp.tile([128, D], BF16, tag="ffb")
            slot_ln_cast(slots_r, ffb)
            ffT = sp.tile([128, KD, 128], BF16, tag="ffT")
            pe_t(ffT, ffb, id_b)
            h1T = sp.tile([128, KH, 128], BF16, tag="h1T")
            for ht in range(KH):
                psh = ps.tile([128, 128], F32, tag="small")
                for kd in range(KD):
                    nc.tensor.matmul(psh, lhsT=w1p_b[:, kd, ts(ht, 128)],
                                     rhs=ffT[:, kd, :],
                                     start=(kd == 0), stop=(kd == KD - 1))
                if has_b1:
                    nc.scalar.activation(out=h1T[:, ht, :], in_=psh,
                                         func=AF.Relu,
                                         bias=b1p_col[:, ht, :], scale=1.0)
                else:
                    nc.scalar.activation(out=h1T[:, ht, :], in_=psh,
                                         func=AF.Relu)
            ps2 = ps.tile([128, D], F32, tag="small")
            for kh in range(KH):
                nc.tensor.matmul(ps2, lhsT=h1T[:, kh, :], rhs=w2_b[:, kh, :],
                                 start=(kh == 0),
                                 stop=(kh == KH - 1 and not has_b2))
            if has_b2:
                nc.tensor.matmul(ps2, lhsT=ones_row, rhs=b2_row,
                                 start=False, stop=True)
            nc.vector.tensor_tensor(out=slots_r, in0=slots_r, in1=ps2,
                                    op=ALU.add)

        for bl in range(ROUND):
            nc.sync.dma_start(out=out_dram[:, rnd * ROUND + bl, :],
                              in_=slots_r[32 * bl:32 * bl + S, :])


def host_fold(w):
    """Fold LN affine params + projection chains on the host (numpy, fp32)."""
    g_in, b_in = w["g_in"], w["b_in"]
    wkp = g_in[:, None] * w["wk"]
    wvp = g_in[:, None] * w["wv"]
    fw = {
        "wkpT": np.ascontiguousarray(wkp.T),
        "wqp": w["g_sl"][:, None] * w["wq"],
        "wvih": wvp @ w["w_ih"],
        "whh": w["w_hh"],
        "w1p": w["g_ff"][:, None] * w["mlp_w1"],
        "w2": w["mlp_w2"],
        "gxbias": (b_in @ w["wv"] + w["bv"]) @ w["w_ih"] + w["b_ih"],
        "ghbias": w["b_hh"],
    }
    fw = {k: np.ascontiguousarray(v).astype(BF) for k, v in fw.items()}
    bkp = (b_in @ w["wk"] + w["bk"]).astype(np.float32)
    bqp = (w["b_sl"] @ w["wq"] + w["bq"]).astype(np.float32)
    b1p = (w["b_ff"] @ w["mlp_w1"] + w["mlp_b1"]).astype(np.float32)
    b2 = w["mlp_b2"].astype(np.float32)
    flags = (bool(np.any(bkp)), bool(np.any(bqp)), bool(np.any(b1p)),
             bool(np.any(b2)))
    if flags[0]:
        fw["bkp"] = bkp
    if flags[1]:
        fw["bqp"] = bqp
    if flags[2]:
        fw["b1p"] = b1p
    if flags[3]:
        fw["b2"] = b2.astype(BF)
    return fw, flags


_NC_CACHE = {}


def get_nc(flags):
    if flags not in _NC_CACHE:
        _NC_CACHE[flags] = _build(flags)
    return _NC_CACHE[flags]


def prepare(inputs, slots, **w):
    """Returns (nc, in_maps) for the 8-core SPMD launch."""
    inputs = np.ascontiguousarray(np.asarray(inputs, np.float32))
    slots = np.ascontiguousarray(np.asarray(slots, np.float32))
    w = {k: np.asarray(v, np.float32) for k, v in w.items()}
    fw, flags = host_fold(w)
    nc = get_nc(flags)
    n_cores = 8
    bs = inputs.shape[0] // n_cores
    in_maps = []
    for c in range(n_cores):
        m = dict(fw)
        m["inputs"] = inputs[c * bs:(c + 1) * bs]
        m["slots"] = np.ascontiguousarray(slots[:, c * bs:(c + 1) * bs, :])
        in_maps.append(m)
    return nc, in_maps


def kernel(inputs, slots, **w):
    nc, in_maps = prepare(inputs, slots, **w)
    res = run_bass_kernel_spmd(nc, in_maps, core_ids=list(range(len(in_maps))))
    out = np.concatenate([r["out"] for r in res.results], axis=1)
    return np.ascontiguousarray(out.astype(np.float32))


if __name__ == "__main__":
    nc = _build((False, False, False, False))
    print("built ok; instructions:", len(nc.inst_map))
